# revision 50
# baseline (speedup 1.0000x reference)
"""Multi-head attention (B=4, S=2048, D=1024, H=16, dk=dv=64) on 8 TRN2 cores.

Sharding: core c = 2*b + hg handles batch b = c//2 and heads
[hg*8, hg*8+8). Each core computes a partial output
(its 8 heads' contribution through Wo); the host adds the two partials
per batch.

Per-core device pipeline (matmul inputs bf16, PSUM accumulation fp32):
  1. Prelude: khT projection (all blocks), vh projection, qhT block 0.
     khT/qhT pair layout: partitions 0-63 = h0's dk, 64-127 = h1's dk.
     vh stored per key-chunk as [128 tok, 8*65] bf16 (64 dv cols + a
     mask/ones col per head, masked keys zeroed).
  2. scores^T per head pair via 64x128 PE row tiling: h0 on tile
     (0,0) (SBUF partitions 0-63), h1 on tile (64,0) (partitions
     64-127); the two matmuls run concurrently on the PE array.
     Output [128 keys, 512 q] fp32 in PSUM, two key-chunks per tile.
  3. exp on ScalarE PSUM->SBUF bf16. The emission is software-
     pipelined so ScalarE never waits: scores(g+1) [or the next
     pair's scores(0)] are emitted between exp(g) and mix(g).
  4. mix^T + softmax sums in one matmul: lhsT = vh_aug [128 keys, 65]
     (col 64 = mask), rhs = exp chunk half [128, 512]; accumulate all
     16 key chunks into one PSUM bank per head (rows 0-64).
  5. normalize: madd = PSUM rows 0-64 -> SBUF fp32 (DVE); sums row 64
     broadcast across partitions with a float32r PE matmul (lhsT =
     e65, K=65, only row 64 ones); reciprocal + multiply (DVE, out
     bf16). h1's tile is DMA-shifted to partitions 64-127 so each
     pair's normalized mix^T is one [128, 512] tile (e on partitions).
  6. out += mixT_norm.T @ Wo: dense K=128 bf16 matmuls accumulating
     over the 4 pairs; DVE evac fp32 -> DMA to HBM. Wo work for block
     qb and the q projection for block qb+1 are deferred thunks run
     inside later pairs' g-loops to fill PE slack under ScalarE.
"""

import numpy as np

B, S, D = 4, 2048, 1024
H, DK, DV = 16, 64, 64
HC = 8          # heads per core
NP = HC // 2    # head pairs per core
NCORES = 8
NC_CHUNKS = D // 128    # 8 contraction chunks over D
NKC = S // 128          # 16 key chunks
NQB = S // 512          # 4 query blocks
NG = NKC // 2           # score/exp groups per pair (2 key chunks each)
VW = HC * 65            # vh storage: 65 cols per head (dv | mask)

_COMPILED = {}

_E65 = np.zeros((128, DV + 1), np.float32)
_E65[64, :] = 1.0


def _build_nc():
    import concourse.tile as tile
    from concourse import bacc, mybir
    from contextlib import ExitStack

    F32 = mybir.dt.float32
    F32R = mybir.dt.float32r
    BF16 = mybir.dt.bfloat16
    EXP = mybir.ActivationFunctionType.Exp

    nc = bacc.Bacc("TRN2", target_bir_lowering=False, debug=False,
                   num_devices=NCORES)

    qT = nc.dram_tensor("qT", [D, S], BF16, kind="ExternalInput").ap()
    kT = nc.dram_tensor("kT", [D, S], BF16, kind="ExternalInput").ap()
    vT = nc.dram_tensor("vT", [D, S], BF16, kind="ExternalInput").ap()
    # wk|wq|wv packed along columns: 3 KiB contiguous HBM rows so the
    # staging DMA uses 3x fewer descriptors than separate tensors
    wkqv = nc.dram_tensor("wkqv", [D, 3 * HC * DK], BF16,
                          kind="ExternalInput").ap()
    wo = nc.dram_tensor("wo", [HC * DV, D], BF16, kind="ExternalInput").ap()
    maskr = nc.dram_tensor("maskr", [128, NKC], F32, kind="ExternalInput").ap()
    e65r = nc.dram_tensor("e65", [128, DV + 1], F32R,
                          kind="ExternalInput").ap()
    out = nc.dram_tensor("out", [S, D], F32, kind="ExternalOutput").ap()

    with tile.TileContext(nc) as tc:
        with ExitStack() as ctx:
            const_pool = ctx.enter_context(tc.tile_pool(name="const", bufs=1))
            w_pool = ctx.enter_context(tc.tile_pool(name="weights", bufs=1))
            act_pool = ctx.enter_context(tc.tile_pool(name="acts", bufs=1))
            stg_pool = ctx.enter_context(tc.tile_pool(name="stg", bufs=1))

            # PSUM pools: 4 + 2 + 2 = 8 banks exactly.
            sc_pool = ctx.enter_context(
                tc.tile_pool(name="scpsum", bufs=2, space="PSUM"))
            mx_pool = ctx.enter_context(
                tc.tile_pool(name="mxpsum", bufs=2, space="PSUM"))
            util_pool = ctx.enter_context(
                tc.tile_pool(name="utpsum", bufs=2, space="PSUM"))

            exp_pool = ctx.enter_context(tc.tile_pool(name="exp", bufs=6))
            norm_pool = ctx.enter_context(tc.tile_pool(name="norm", bufs=8))
            tmp_pool = ctx.enter_context(tc.tile_pool(name="tmp", bufs=2))
            out_pool = ctx.enter_context(tc.tile_pool(name="outsb", bufs=4))

            mask_sb = const_pool.tile([128, NKC], F32)
            nc.sync.dma_start(mask_sb[:], maskr[:])
            ones_sb = const_pool.tile([128, 64], BF16)
            nc.vector.memset(ones_sb[:], 1.0)
            # bcast helper: only row 64 ones -> out rows = sums row replicated
            e65_sb = const_pool.tile([128, DV + 1], F32R)
            nc.sync.dma_start(e65_sb[:], e65r[:])

            # per chunk c: [wk_c | wq_c | wv_c], each 512 cols
            wkqv_sb = w_pool.tile([128, NC_CHUNKS * 1536], BF16, tag="wkqv")
            wo_sb = w_pool.tile([128, NP * 1024], BF16, tag="wo")
            for c in range(NC_CHUNKS):
                nc.sync.dma_start(wkqv_sb[:, c * 1536:(c + 1) * 1536],
                                  wkqv[c * 128:(c + 1) * 128, :])

            def wk_col(c, lo, hi):
                return wkqv_sb[:, c * 1536 + lo:c * 1536 + hi]

            def wq_col(c, lo, hi):
                return wkqv_sb[:, c * 1536 + 512 + lo:c * 1536 + 512 + hi]

            def wv_col(c, lo, hi):
                return wkqv_sb[:, c * 1536 + 1024 + lo:c * 1536 + 1024 + hi]

            # persistent activations
            qhT = [act_pool.tile([128, S], BF16, tag=f"qhT{p}", name=f"qhT{p}")
                   for p in range(NP)]
            khT0 = [act_pool.tile([128, S], BF16, tag=f"khT0{p}",
                                  name=f"khT0{p}") for p in range(NP)]
            khT1 = [act_pool.tile([128, S], BF16, tag=f"khT1{p}",
                                  name=f"khT1{p}") for p in range(NP)]
            for p in range(NP):
                nc.vector.memset(khT0[p][64:128, :], 0.0)
                nc.vector.memset(khT1[p][0:64, :], 0.0)
            vhs = [act_pool.tile([128, VW], BF16, tag=f"vh{t}", name=f"vh{t}")
                   for t in range(NKC)]

            # Wide staging tiles: full-row HBM reads give 2-4 KiB DMA
            # descriptors (the staging path is descriptor-rate limited).
            # kT is staged whole; qT and vT are staged in halves and the
            # second half overwrites the first once its readers finish.
            kst = [stg_pool.tile([128, S], BF16, tag=f"kst{c}",
                                 name=f"kst{c}") for c in range(NC_CHUNKS)]
            qst = [stg_pool.tile([128, 1024], BF16, tag=f"qst{c}",
                                 name=f"qst{c}") for c in range(NC_CHUNKS)]
            vst = [stg_pool.tile([128, 1024], BF16, tag=f"vst{c}",
                                 name=f"vst{c}") for c in range(NC_CHUNKS)]

            def proj_k(kb, p):
                ps = util_pool.tile([128, 512], F32, tag="ut")
                for c in range(NC_CHUNKS):
                    nc.tensor.matmul(
                        ps[:],
                        lhsT=wk_col(c, p * 128, (p + 1) * 128),
                        rhs=kst[c][:, kb * 512:(kb + 1) * 512],
                        start=(c == 0), stop=(c == NC_CHUNKS - 1))
                qsl = slice(kb * 512, (kb + 1) * 512)
                nc.vector.tensor_copy(khT0[p][0:64, qsl], ps[0:64, :])
                nc.vector.tensor_copy(khT1[p][64:128, qsl], ps[64:128, :])

            def proj_q(qb, p):
                ps = util_pool.tile([128, 512], F32, tag="ut")
                h = qb % 2
                for c in range(NC_CHUNKS):
                    nc.tensor.matmul(
                        ps[:],
                        lhsT=wq_col(c, p * 128, (p + 1) * 128),
                        rhs=qst[c][:, h * 512:(h + 1) * 512],
                        start=(c == 0), stop=(c == NC_CHUNKS - 1))
                nc.vector.tensor_copy(qhT[p][:, qb * 512:(qb + 1) * 512],
                                      ps[:])

            def vproj_t(t):
                """Project key-chunk t of v into vhs[t] (mask folded)."""
                i = t % 8
                ps = util_pool.tile([128, 512], F32, tag="ut")
                for c in range(NC_CHUNKS):
                    nc.tensor.matmul(
                        ps[:],
                        lhsT=vst[c][:, i * 128:(i + 1) * 128],
                        rhs=wv_col(c, 0, 512),
                        start=(c == 0), stop=(c == NC_CHUNKS - 1))
                dst_dv = vhs[t][:, 0:VW].rearrange(
                    "p (h x) -> p h x", x=65)[:, :, 0:DV]
                src_dv = ps[:].rearrange("p (h x) -> p h x", x=DV)
                nc.vector.tensor_scalar_mul(dst_dv, src_dv,
                                            mask_sb[:, t:t + 1])
                dst_m = vhs[t][:, 0:VW].rearrange(
                    "p (h x) -> p h x", x=65)[:, :, DV:DV + 1]
                src_m = ones_sb[:, 0:HC].rearrange("p (h x) -> p h x", x=1)
                nc.vector.tensor_scalar_mul(dst_m, src_m,
                                            mask_sb[:, t:t + 1])

            # ---- prelude staging + first projections ----
            # DMA queue order matters: vT's first half lands first so
            # vproj warms the PE while kT is still arriving.
            for c in range(NC_CHUNKS):
                nc.sync.dma_start(vst[c][:],
                                  vT[c * 128:(c + 1) * 128, 0:1024])
            for c in range(NC_CHUNKS):
                nc.sync.dma_start(kst[c][:], kT[c * 128:(c + 1) * 128, :])
            for c in range(NC_CHUNKS):
                nc.sync.dma_start(qst[c][:],
                                  qT[c * 128:(c + 1) * 128, 0:1024])
            for t in range(8):
                vproj_t(t)
            # second half of v overwrites the first (waits on its readers)
            for c in range(NC_CHUNKS):
                nc.sync.dma_start(vst[c][:],
                                  vT[c * 128:(c + 1) * 128, 1024:2048])
            proj_k(0, 0)
            proj_k(1, 0)
            proj_k(2, 0)
            proj_k(3, 0)
            for t in range(8, NKC):
                vproj_t(t)
            proj_q(0, 0)

            # ---- attention + output projection ----
            def emit_scores(qb, p, kc, scs):
                """Scores for one key chunk, both heads (full PE array;
                khT0/khT1 zero-padded halves select the head).

                h0's [128 keys, 512 q] goes to the tile's first bank,
                h1's to its second. sc_pool bufs=2 double-buffers: the
                write of scores(kc+1) only waits on exp(kc-1)."""
                qful = qhT[p][:, qb * 512:(qb + 1) * 512]
                sc = sc_pool.tile([128, 1024], F32, tag="sc")
                ksl = slice(kc * 128, (kc + 1) * 128)
                nc.tensor.matmul(
                    sc[:, 0:512],
                    lhsT=khT0[p][:, ksl], rhs=qful,
                    start=True, stop=True)
                nc.tensor.matmul(
                    sc[:, 512:1024],
                    lhsT=khT1[p][:, ksl], rhs=qful,
                    start=True, stop=True)
                scs.append(sc)

            def emit_wo_tt(qb, normT, tt):
                """One eighth of the Wo projection for query block qb."""
                tt4, dh = tt // 2, tt % 2
                wps = util_pool.tile([128, 512], F32, tag="ut")
                for p in range(NP):
                    nc.tensor.matmul(
                        wps[:],
                        lhsT=normT[p][:, tt4 * 128:(tt4 + 1) * 128],
                        rhs=wo_sb[:, p * 1024 + dh * 512:
                                  p * 1024 + (dh + 1) * 512],
                        start=(p == 0), stop=(p == NP - 1))
                osb = out_pool.tile([128, 512], F32, tag="osb")
                nc.vector.tensor_copy(osb[:], wps[:])
                nc.sync.dma_start(
                    out[qb * 512 + tt4 * 128:qb * 512 + (tt4 + 1) * 128,
                        dh * 512:(dh + 1) * 512], osb[:])

            # Deferred PE work, deadline-paced into the attention windows.
            # Window index w = qb*64 + p*16 + kc counts exp instructions;
            # thunks run inside window slack so ScalarE stays fed.
            pending = []    # (deadline_window, emit_fn)

            def pump(w):
                pending.sort(key=lambda x: x[0])
                ran = 0
                while pending:
                    dl, fn = pending[0]
                    if dl <= w + 2:
                        pending.pop(0)
                        fn()
                        ran += 1
                    elif ran == 0 and dl <= w + 10:
                        pending.pop(0)
                        fn()
                        ran += 1
                    else:
                        break

            for p in range(1, NP):
                for kb in range(NQB):
                    pending.append(
                        (16 * p + 4 * kb - 4,
                         lambda kb=kb, p=p: proj_k(kb, p)))
                pending.append(
                    (16 * p - 4, lambda p=p: proj_q(0, p)))
            for nqb in range(1, NQB):
                for p in range(NP):
                    dl = 64 * nqb - 10 if p == 0 else 64 * nqb + 16 * p - 14
                    pending.append(
                        (dl, lambda nqb=nqb, p=p: proj_q(nqb, p)))

            def stage_wo():
                for p in range(NP):
                    nc.sync.dma_start(wo_sb[:, p * 1024:(p + 1) * 1024],
                                      wo[p * 128:(p + 1) * 128, :])
            pending.append((20, stage_wo))

            # restage qT's second half (query blocks 2-3) once the
            # block-1 projections are done reading the first half
            def restage_q():
                for c in range(NC_CHUNKS):
                    nc.sync.dma_start(
                        qst[c][:], qT[c * 128:(c + 1) * 128, 1024:2048])
            pending.append((112, restage_q))

            cur_scs = []
            emit_scores(0, 0, 0, cur_scs)

            for qb in range(NQB):
                normT = []
                for p in range(NP):
                    h0, h1 = 2 * p, 2 * p + 1
                    l0 = slice(h0 * 65, h0 * 65 + 65)
                    l1 = slice(h1 * 65, h1 * 65 + 65)
                    scs = cur_scs
                    mixP = mx_pool.tile([128, 512], F32, tag="mx")
                    mixR = mx_pool.tile([128, 512], F32, tag="mx")
                    for kc in range(NKC):
                        w = qb * 64 + p * 16 + kc
                        ex = exp_pool.tile([128, 1024], BF16, tag="exp")
                        nc.scalar.activation(ex[:], scs[kc][:], EXP)
                        # keep ScalarE fed: emit the next scores chunk now
                        if kc + 1 < NKC:
                            emit_scores(qb, p, kc + 1, scs)
                        elif (qb, p) != (NQB - 1, NP - 1):
                            nqb, np_ = (qb, p + 1) if p + 1 < NP else (qb + 1, 0)
                            cur_scs = []
                            emit_scores(nqb, np_, 0, cur_scs)
                        pump(w)
                        va = vhs[kc]
                        st = (kc == 0)
                        sp = (kc == NKC - 1)
                        nc.tensor.matmul(
                            mixP[0:65, :],
                            lhsT=va[:, l0], rhs=ex[:, 0:512],
                            start=st, stop=sp)
                        nc.tensor.matmul(
                            mixR[0:65, :],
                            lhsT=va[:, l1], rhs=ex[:, 512:1024],
                            start=st, stop=sp)
                    # normalize (sums broadcast via f32r PE matmul, K=65)
                    nt = norm_pool.tile([128, 512], BF16, tag="norm")
                    normT.append(nt)
                    madd0 = tmp_pool.tile([128, 512], F32R, tag="madd")
                    madd1 = tmp_pool.tile([128, 512], F32R, tag="madd")
                    nc.vector.tensor_copy(madd0[0:65, :], mixP[0:65, :])
                    nc.vector.tensor_copy(madd1[0:65, :], mixR[0:65, :])
                    bc0 = util_pool.tile([128, 512], F32, tag="ut")
                    bc1 = util_pool.tile([128, 512], F32, tag="ut")
                    nc.tensor.matmul(
                        bc0[0:DV + 1, :],
                        lhsT=e65_sb[0:DV + 1, 0:DV + 1],
                        rhs=madd0[0:DV + 1, :],
                        start=True, stop=True)
                    nc.tensor.matmul(
                        bc1[0:DV + 1, :],
                        lhsT=e65_sb[0:DV + 1, 0:DV + 1],
                        rhs=madd1[0:DV + 1, :],
                        start=True, stop=True)
                    rec0 = tmp_pool.tile([64, 512], F32, tag="rec")
                    rec1 = tmp_pool.tile([64, 512], F32, tag="rec")
                    nc.vector.reciprocal_approx_fast(rec0[:], bc0[0:64, :])
                    nc.vector.reciprocal_approx_fast(rec1[:], bc1[0:64, :])
                    nc.vector.tensor_mul(nt[0:64, :], madd0[0:64, :],
                                         rec0[:])
                    sh1 = tmp_pool.tile([64, 512], BF16, tag="sh1")
                    nc.vector.tensor_mul(sh1[:], madd1[0:64, :],
                                         rec1[:])
                    nc.sync.dma_start(nt[64:128, :], sh1[:])

                # Wo for this block runs inside the next block's windows
                for tt in range(8):
                    pending.append(
                        (64 * (qb + 1) + 24 + 4 * tt,
                         lambda qb=qb, normT=normT, tt=tt: emit_wo_tt(
                             qb, normT, tt)))

            # drain whatever is left (last block's Wo)
            pending.sort(key=lambda x: x[0])
            for _, fn in pending:
                fn()

    nc.compile()
    return nc


def _get_nc():
    if "nc" not in _COMPILED:
        _COMPILED["nc"] = _build_nc()
    return _COMPILED["nc"]


def _shard_inputs(q, k, v, mask, Wq, Wk, Wv, Wo):
    """Build the per-core input maps (host-side layout prep)."""
    import ml_dtypes

    bf16 = ml_dtypes.bfloat16
    in_maps = []
    maskf = np.asarray(mask).astype(np.float32)
    q = np.asarray(q, np.float32)
    k = np.asarray(k, np.float32)
    v = np.asarray(v, np.float32)
    Wq = np.asarray(Wq, np.float32)
    Wk = np.asarray(Wk, np.float32)
    Wv = np.asarray(Wv, np.float32)
    Wo = np.asarray(Wo, np.float32)
    scale = np.float32(1.0 / np.sqrt(DK))
    for c in range(NCORES):
        b, hg = c // 2, c % 2
        hs = hg * HC
        # head-major col blocks; fold 1/sqrt(dk) into Wq; pack [wk|wq|wv]
        wkh = Wk[hs:hs + HC].transpose(1, 0, 2).reshape(D, HC * DK)
        wqh = Wq[hs:hs + HC].transpose(1, 0, 2).reshape(D, HC * DK) * scale
        wvh = Wv[hs:hs + HC].transpose(1, 0, 2).reshape(D, HC * DV)
        m = {
            "qT": np.ascontiguousarray(q[b].T).astype(bf16),
            "kT": np.ascontiguousarray(k[b].T).astype(bf16),
            "vT": np.ascontiguousarray(v[b].T).astype(bf16),
            "wkqv": np.ascontiguousarray(
                np.concatenate([wkh, wqh, wvh], axis=1)).astype(bf16),
            "wo": np.ascontiguousarray(Wo[hs * DV:(hs + HC) * DV]).astype(bf16),
            "maskr": np.ascontiguousarray(
                maskf[b].reshape(NKC, 128).T).astype(np.float32),
            "e65": _E65,
        }
        in_maps.append(m)
    return in_maps


def kernel(q, k, v, mask, Wq, Wk, Wv, Wo, _trace=False):
    from concourse.bass_utils import run_bass_kernel_spmd

    nc = _get_nc()
    in_maps = _shard_inputs(q, k, v, mask, Wq, Wk, Wv, Wo)
    res = run_bass_kernel_spmd(nc, in_maps, list(range(NCORES)),
                               trace=_trace)
    out = np.zeros((B, S, D), np.float32)
    for c in range(NCORES):
        out[c // 2] += res.results[c]["out"]
    if _trace:
        _COMPILED["last_result"] = res
    return out
